# revision 14
# baseline (speedup 1.0000x reference)
"""Multi-head self-attention block (B=2, S=2048, D=1024, H=16) on 8 TRN2 cores.

Sharding: 2-way data-parallel over batch x 4-way tensor-parallel over heads.
Core c handles batch b=c//4 with group rank g=c%4 (heads 4g..4g+4) and owns
output rows [512g, 512(g+1)) of its batch (delivered by a ReduceScatter of
the partial out-projection over the 4-core batch group).

Self-contained: hardcodes all shapes; builds the Bass program once.
"""

import os
import sys

sys.path.insert(0, "/opt/trn_rl_repo")

import numpy as np
import ml_dtypes

import concourse.bass as bass
import concourse.tile as tile
from concourse import bacc, mybir
from concourse.bass_utils import run_bass_kernel_spmd
from concourse.masks import make_identity

B, S, D, H = 2, 2048, 1024, 16
A = D // H  # 64
NCORES = 8
G = 4  # cores per batch group
HL = H // G  # local heads per core = 4
M_QK = 2 * HL * A  # 512 rows of Q_T+K_T per core
QB = S // G  # 512: output q-block per core
EPS = 1e-3
GROUPS = [[0, 1, 2, 3], [4, 5, 6, 7]]

f32 = mybir.dt.float32
f32r = mybir.dt.float32r
bf16 = mybir.dt.bfloat16

AF = mybir.ActivationFunctionType
OP = mybir.AluOpType

_CACHE = {}


def _build():
    nc = bacc.Bacc("TRN2", target_bir_lowering=False, debug=False, num_devices=NCORES)

    # ---- I/O ----
    embT_d = nc.dram_tensor("embT", [D, S], f32r, kind="ExternalInput")
    embres_d = nc.dram_tensor("embres", [QB, D], f32, kind="ExternalInput")
    maskT_d = nc.dram_tensor("maskT", [S, S], bf16, kind="ExternalInput")
    wqk_d = nc.dram_tensor("wqk", [D, M_QK], f32r, kind="ExternalInput")
    wv_d = nc.dram_tensor("wv", [D, HL * A], f32r, kind="ExternalInput")
    bqk_d = nc.dram_tensor("bqk", [128, 4], f32, kind="ExternalInput")
    bv_d = nc.dram_tensor("bv", [1, HL * A], f32r, kind="ExternalInput")
    ones_d = nc.dram_tensor("ones", [1, 128], f32r, kind="ExternalInput")
    wout_d = nc.dram_tensor("wout", [128, 2, D], bf16, kind="ExternalInput")
    bout_d = nc.dram_tensor("bout", [1, D], f32, kind="ExternalInput")
    gamma_d = nc.dram_tensor("gamma", [1, D], f32, kind="ExternalInput")
    beta_d = nc.dram_tensor("beta", [1, D], f32, kind="ExternalInput")
    out_d = nc.dram_tensor("out", [QB, D], f32, kind="ExternalOutput")

    with tile.TileContext(nc) as tc:
        with (
            tc.tile_pool(name="big", bufs=1) as big,  # embT then maskT (64KB/p slot)
            tc.tile_pool(name="persist", bufs=1) as persist,
            tc.tile_pool(name="probs", bufs=2) as probsp,
            tc.tile_pool(name="work", bufs=2) as work,
            tc.tile_pool(name="psA", bufs=2, space="PSUM") as psA,  # 512-f32 matmuls
            tc.tile_pool(name="psS", bufs=2, space="PSUM") as psS,  # scores bf16
            tc.tile_pool(name="psB", bufs=2, space="PSUM") as psB,  # pv / transpose
            tc.tile_pool(name="dram", bufs=1, space="DRAM") as dram,
        ):
            # ---------- constants ----------
            ident = persist.tile([128, 128], bf16)
            make_identity(nc, ident)
            ones_r = persist.tile([1, 128], f32r)
            nc.sync.dma_start(out=ones_r, in_=ones_d[:, :])
            eps_sb = persist.tile([128, 1], f32)
            nc.vector.memset(eps_sb, EPS)

            # ---------- load weights ----------
            wqk_sb = persist.tile([128, 8, M_QK], f32r)
            wv_sb = persist.tile([128, 8, HL * A], f32r)
            for kt in range(8):
                nc.sync.dma_start(out=wqk_sb[:, kt, :], in_=wqk_d[kt * 128 : (kt + 1) * 128, :])
                nc.sync.dma_start(out=wv_sb[:, kt, :], in_=wv_d[kt * 128 : (kt + 1) * 128, :])
            bqk_sb = persist.tile([128, 4], f32)
            nc.sync.dma_start(out=bqk_sb, in_=bqk_d[:, :])
            bv_sb = persist.tile([1, HL * A], f32r)
            nc.sync.dma_start(out=bv_sb, in_=bv_d[:, :])
            wout_sb = persist.tile([128, 2, D], bf16)
            nc.sync.dma_start(out=wout_sb, in_=wout_d[:, :, :])

            # ---------- load embT (shares "big" slot with maskT later) ----------
            embT_sb = big.tile([128, 8, S], f32r, tag="bigslot")
            for kt in range(8):
                nc.sync.dma_start(out=embT_sb[:, kt, :], in_=embT_d[kt * 128 : (kt + 1) * 128, :])

            # ---------- QKV projection ----------
            # Q_T/K_T: [m, s] for m in 4 chunks of 128 (Q h0-1, Q h2-3, K h0-1, K h2-3)
            qk_sb = persist.tile([128, 4, S], bf16)
            for mc in range(4):
                for sc in range(4):
                    ps = psA.tile([128, 512], f32, tag="mm512")
                    for kt in range(8):
                        nc.tensor.matmul(
                            ps[:],
                            wqk_sb[:, kt, mc * 128 : (mc + 1) * 128],
                            embT_sb[:, kt, sc * 512 : (sc + 1) * 512],
                            start=(kt == 0),
                            stop=(kt == 7),
                        )
                    nc.scalar.activation(
                        out=qk_sb[:, mc, sc * 512 : (sc + 1) * 512],
                        in_=ps[:],
                        func=AF.Identity,
                        bias=bqk_sb[:, mc : mc + 1],
                        scale=1.0,
                    )

            # V: [s, (h, 1+a)] bf16 with a leading ones column per head (sumexp trick)
            v_sb = persist.tile([128, 16, HL, 1 + A], bf16)
            nc.vector.memset(v_sb, 1.0)
            for st in range(16):
                ps = psA.tile([128, HL * A], f32, tag="mm512")
                for kt in range(8):
                    nc.tensor.matmul(
                        ps[:],
                        embT_sb[:, kt, st * 128 : (st + 1) * 128],
                        wv_sb[:, kt, :],
                        start=(kt == 0),
                        stop=False,
                    )
                nc.tensor.matmul(
                    ps[:], ones_r[:, :], bv_sb[:, :], start=False, stop=True
                )
                nc.vector.tensor_copy(
                    out=v_sb[:, st, :, 1:],
                    in_=ps.rearrange("p (h a) -> p h a", h=HL),
                )

            # ---------- mask (reuses the embT slot; waits for last embT read) ----------
            mask_sb = big.tile([128, 16, S], bf16, tag="bigslot")
            for kb in range(16):
                nc.sync.dma_start(out=mask_sb[:, kb, :], in_=maskT_d[kb * 128 : (kb + 1) * 128, :])

            # ---------- attention units: (q-quarter, head) ----------
            x_sb = persist.tile([128, 16, HL * A], bf16)  # x[q, (h a)] per q-tile
            for quarter in range(4):
                qoff = quarter * 512
                for h in range(4):
                    kslc = slice(64 * (h % 2), 64 * (h % 2) + 64)
                    kmc = 2 + h // 2
                    qmc = h // 2
                    probs = probsp.tile([128, 16, 512], bf16, tag="probs")
                    for j in range(8):  # kb pairs
                        ps_s = psS.tile([128, 2, 512], f32, tag="score")
                        for kk in range(2):
                            kb = 2 * j + kk
                            nc.tensor.matmul(
                                ps_s[:, kk, :],
                                qk_sb[kslc, kmc, kb * 128 : (kb + 1) * 128],
                                qk_sb[kslc, qmc, qoff : qoff + 512],
                                start=True,
                                stop=True,
                            )
                        nc.scalar.activation(
                            out=probs[:, 2 * j : 2 * j + 2, :],
                            in_=ps_s[:, :, :],
                            func=AF.Exp,
                            scale=0.125,
                        )
                    for j in range(4):
                        nc.vector.tensor_tensor(
                            probs[:, 4 * j : 4 * j + 4, :],
                            probs[:, 4 * j : 4 * j + 4, :],
                            mask_sb[:, 4 * j : 4 * j + 4, qoff : qoff + 512],
                            OP.mult,
                        )
                    for qq in range(4):
                        qt = quarter * 4 + qq
                        ps_pv = psB.tile([128, 1 + A], f32, tag="small")
                        for kb in range(16):
                            nc.tensor.matmul(
                                ps_pv[:],
                                probs[:, kb, qq * 128 : (qq + 1) * 128],
                                v_sb[:, kb, h, :],
                                start=(kb == 0),
                                stop=(kb == 15),
                            )
                        recip = work.tile([128, 1], f32, tag="recip")
                        nc.vector.reciprocal(recip, ps_pv[:, 0:1])
                        nc.vector.tensor_scalar(
                            x_sb[:, qt, h * A : (h + 1) * A],
                            ps_pv[:, 1:],
                            recip,
                            None,
                            OP.mult,
                        )

            # ---------- transpose x -> xT [(h a), q] ----------
            xT_sb = persist.tile([128, 2, S], bf16)
            for qt in range(16):
                for cb in range(2):
                    ps_t = psB.tile([128, 128], bf16, tag="small")
                    nc.tensor.transpose(
                        ps_t[:], x_sb[:, qt, cb * 128 : (cb + 1) * 128], ident[:]
                    )
                    nc.vector.tensor_copy(
                        out=xT_sb[:, cb, qt * 128 : (qt + 1) * 128], in_=ps_t[:]
                    )

            # ---------- out-projection (partial over local heads) + ReduceScatter ----------
            rs_in = [
                dram.tile([S, 512], f32, name=f"rsin{k}", tag=f"rsin{k}")
                for k in range(2)
            ]
            rs_out = [
                dram.tile([QB, 512], f32, name=f"rsout{k}", tag=f"rsout{k}")
                for k in range(2)
            ]
            for dc in range(2):
                for qt in range(16):
                    ps = psA.tile([128, 512], f32, tag="mm512")
                    for ct in range(2):
                        nc.tensor.matmul(
                            ps[:],
                            xT_sb[:, ct, qt * 128 : (qt + 1) * 128],
                            wout_sb[:, ct, dc * 512 : (dc + 1) * 512],
                            start=(ct == 0),
                            stop=(ct == 1),
                        )
                    oe = work.tile([128, 512], f32, tag="oevict", bufs=3)
                    nc.any.tensor_copy(out=oe[:], in_=ps[:])
                    nc.sync.dma_start(
                        out=rs_in[dc][qt * 128 : (qt + 1) * 128, :], in_=oe[:]
                    )
                nc.gpsimd.collective_compute(
                    "ReduceScatter",
                    OP.add,
                    replica_groups=GROUPS,
                    ins=[rs_in[dc][:, :].opt()],
                    outs=[rs_out[dc][:, :].opt()],
                )

            # ---------- residual + layernorm on own q-block ----------
            boutbc = persist.tile([128, D], f32)
            gammabc = persist.tile([128, D], f32)
            betabc = persist.tile([128, D], f32)
            for t, dr in ((boutbc, bout_d), (gammabc, gamma_d), (betabc, beta_d)):
                src = dr[:, :]
                bc = bass.AP(tensor=src.tensor, offset=src.offset, ap=[[0, 128], src.ap[1]])
                nc.sync.dma_start(out=t[:], in_=bc)

            for q4 in range(4):
                rsl = slice(q4 * 128, (q4 + 1) * 128)
                y = work.tile([128, D], f32, tag="y")
                for dc in range(2):
                    nc.sync.dma_start(
                        out=y[:, dc * 512 : (dc + 1) * 512], in_=rs_out[dc][rsl, :]
                    )
                er = work.tile([128, D], f32, tag="er")
                nc.sync.dma_start(out=er[:], in_=embres_d[rsl, :])
                nc.vector.tensor_tensor(y[:], y[:], er[:], OP.add)
                nc.vector.tensor_tensor(y[:], y[:], boutbc[:], OP.add)

                stats = work.tile([128, 2, nc.vector.BN_STATS_DIM], f32, tag="stats")
                for sg in range(2):
                    nc.vector.bn_stats(
                        out=stats[:, sg, :], in_=y[:, sg * 512 : (sg + 1) * 512]
                    )
                mv = work.tile([128, nc.vector.BN_AGGR_DIM], f32, tag="mv")
                nc.vector.bn_aggr(out=mv[:], in_=stats[:])
                rstd = work.tile([128, 1], f32, tag="rstd")
                nc.scalar.activation(
                    out=rstd[:], in_=mv[:, 1:2], func=AF.Sqrt, bias=eps_sb[:], scale=1.0
                )
                nc.vector.reciprocal(rstd[:], rstd[:])
                nc.vector.tensor_scalar(
                    y[:], y[:], mv[:, 0:1], rstd[:], OP.subtract, OP.mult
                )
                o = work.tile([128, D], f32, tag="er")
                nc.vector.tensor_tensor(o[:], y[:], gammabc[:], OP.mult)
                nc.vector.tensor_tensor(o[:], o[:], betabc[:], OP.add)
                nc.sync.dma_start(out=out_d[rsl, :], in_=o[:])

    nc.compile()
    return nc


def _prep_inputs(embeddings, attention_mask, W_qkv, b_qkv, W_out, b_out, ln_gamma, ln_beta):
    emb = np.asarray(embeddings, dtype=np.float32)
    mask = np.asarray(attention_mask)
    W_qkv = np.asarray(W_qkv, dtype=np.float32)
    b_qkv = np.asarray(b_qkv, dtype=np.float32)
    W_out = np.asarray(W_out, dtype=np.float32)
    b_out = np.asarray(b_out, dtype=np.float32)
    gamma = np.asarray(ln_gamma, dtype=np.float32).reshape(1, D)
    beta = np.asarray(ln_beta, dtype=np.float32).reshape(1, D)

    in_maps = []
    for c in range(NCORES):
        b = c // G
        g = c % G
        hs = g * HL * A  # 256g
        embT = np.ascontiguousarray(emb[b].T)
        maskT = np.ascontiguousarray(mask[b].T).astype(ml_dtypes.bfloat16)
        wqk = np.ascontiguousarray(
            np.concatenate([W_qkv[:, hs : hs + 256], W_qkv[:, D + hs : D + hs + 256]], axis=1)
        )
        wv = np.ascontiguousarray(W_qkv[:, 2 * D + hs : 2 * D + hs + 256])
        bqk = np.concatenate([b_qkv[hs : hs + 256], b_qkv[D + hs : D + hs + 256]])
        bqk = np.ascontiguousarray(bqk.reshape(4, 128).T)
        bv = np.ascontiguousarray(b_qkv[2 * D + hs : 2 * D + hs + 256].reshape(1, 256))
        wout = np.ascontiguousarray(
            W_out[hs : hs + 256, :].reshape(2, 128, D).transpose(1, 0, 2)
        ).astype(ml_dtypes.bfloat16)
        in_maps.append(
            {
                "embT": embT,
                "embres": np.ascontiguousarray(emb[b, g * QB : (g + 1) * QB, :]),
                "maskT": maskT,
                "wqk": wqk,
                "wv": wv,
                "bqk": bqk,
                "bv": bv,
                "ones": np.ones((1, 128), dtype=np.float32),
                "wout": wout,
                "bout": b_out.reshape(1, D),
                "gamma": gamma,
                "beta": beta,
            }
        )
    return in_maps


def _run(inputs, trace=False, **kw):
    if "nc" not in _CACHE:
        _CACHE["nc"] = _build()
    nc = _CACHE["nc"]
    in_maps = _prep_inputs(**inputs)
    res = run_bass_kernel_spmd(nc, in_maps, list(range(NCORES)), trace=trace, **kw)
    out = np.empty((B, S, D), dtype=np.float32)
    for c in range(NCORES):
        b, g = c // G, c % G
        out[b, g * QB : (g + 1) * QB, :] = res.results[c]["out"]
    return out, res


def kernel(**inputs):
    out, _ = _run(inputs, trace=False)
    return out


# revision 24
# speedup vs baseline: 1.0940x; 1.0940x over previous
"""Multi-head self-attention block (B=2, S=2048, D=1024, H=16) on 8 TRN2 cores.

Sharding: 2-way data-parallel over batch x 4-way tensor-parallel over heads.
Core c handles batch b=c//4 with group rank g=c%4 (heads 4g..4g+4). The
out-projection partials are combined with one bf16 ReduceScatter per
q-quarter over the 4-core batch group, so core g owns output rows
[512q + 128g, 512q + 128(g+1)) for q in 0..4 — collectives overlap the
remaining attention quarters instead of forming a serial tail.

Self-contained: hardcodes all shapes; builds the Bass program once.
"""

import os
import sys

sys.path.insert(0, "/opt/trn_rl_repo")

import numpy as np
import ml_dtypes

import concourse.bass as bass
import concourse.tile as tile
from concourse import bacc, mybir
from concourse.bass_utils import run_bass_kernel_spmd

B, S, D, H = 2, 2048, 1024, 16
A = D // H  # 64
NCORES = 8
G = 4  # cores per batch group
HL = H // G  # local heads per core = 4
M_QK = 2 * HL * A  # 512 rows of Q_T+K_T per core
QB = S // G  # 512
EPS = 1e-3
GROUPS = [[0, 1, 2, 3], [4, 5, 6, 7]]

f32 = mybir.dt.float32
f32r = mybir.dt.float32r
bf16 = mybir.dt.bfloat16

AF = mybir.ActivationFunctionType
OP = mybir.AluOpType

_CACHE = {}


def _build():
    nc = bacc.Bacc("TRN2", target_bir_lowering=False, debug=False, num_devices=NCORES)

    # ---- I/O ----
    embT_d = nc.dram_tensor("embT", [D, S], f32r, kind="ExternalInput")
    embres_d = nc.dram_tensor("embres", [QB, D], f32, kind="ExternalInput")
    maskT_d = nc.dram_tensor("maskT", [S, S], bf16, kind="ExternalInput")
    wqk_d = nc.dram_tensor("wqk", [D, M_QK], f32r, kind="ExternalInput")
    wv_d = nc.dram_tensor("wv", [D, HL * A], f32r, kind="ExternalInput")
    bqk_d = nc.dram_tensor("bqk", [128, 4], f32, kind="ExternalInput")
    bv_d = nc.dram_tensor("bv", [1, HL * A], f32r, kind="ExternalInput")
    ones_d = nc.dram_tensor("ones", [1, 128], f32r, kind="ExternalInput")
    # W_out shard as [a=64, local head, D]
    wout_d = nc.dram_tensor("wout", [64, HL, D], bf16, kind="ExternalInput")
    bout_d = nc.dram_tensor("bout", [1, D], f32, kind="ExternalInput")
    gamma_d = nc.dram_tensor("gamma", [1, D], f32, kind="ExternalInput")
    beta_d = nc.dram_tensor("beta", [1, D], f32, kind="ExternalInput")
    out_d = nc.dram_tensor("out", [QB, D], f32, kind="ExternalOutput")

    with tile.TileContext(nc) as tc:
        with (
            tc.tile_pool(name="big", bufs=1) as big,  # embT then maskT (64KB/p slot)
            tc.tile_pool(name="persist", bufs=1) as persist,
            tc.tile_pool(name="probs", bufs=2) as probsp,
            tc.tile_pool(name="work", bufs=2) as work,
            tc.tile_pool(name="psA", bufs=2, space="PSUM") as psA,  # 1-bank f32 mm
            tc.tile_pool(name="psS", bufs=2, space="PSUM") as psS,  # scores (2 banks)
            tc.tile_pool(name="psB", bufs=2, space="PSUM") as psB,  # pv xT (1 bank)
            tc.tile_pool(name="dram", bufs=1, space="DRAM") as dram,
        ):
            # ---------- embT first: it gates the QKV critical path ----------
            embT_sb = big.tile([128, 8, S], f32r, tag="bigslot")
            for kt in range(8):
                nc.sync.dma_start(out=embT_sb[:, kt, :], in_=embT_d[kt * 128 : (kt + 1) * 128, :])

            # ---------- weights / constants ----------
            wqk_sb = persist.tile([128, 8, M_QK], f32r)
            wv_sb = persist.tile([128, 8, HL * A], f32r)
            for kt in range(8):
                nc.sync.dma_start(out=wqk_sb[:, kt, :], in_=wqk_d[kt * 128 : (kt + 1) * 128, :])
                nc.sync.dma_start(out=wv_sb[:, kt, :], in_=wv_d[kt * 128 : (kt + 1) * 128, :])
            bqk_sb = persist.tile([128, 4], f32)
            nc.sync.dma_start(out=bqk_sb, in_=bqk_d[:, :])
            bv_sb = persist.tile([1, HL * A], f32r)
            nc.sync.dma_start(out=bv_sb, in_=bv_d[:, :])
            ones_r = persist.tile([1, 128], f32r)
            nc.sync.dma_start(out=ones_r, in_=ones_d[:, :])
            ones64 = persist.tile([65, 128], f32r)
            nc.sync.dma_start(out=ones64[64:65, :], in_=ones_d[:, :])
            wout_sb = persist.tile([64, HL, D], bf16)
            nc.sync.dma_start(out=wout_sb, in_=wout_d[:, :, :])
            eps_sb = persist.tile([128, 1], f32)
            nc.vector.memset(eps_sb, EPS)
            boutbc = persist.tile([128, D], f32)
            gammabc = persist.tile([128, D], f32)
            betabc = persist.tile([128, D], f32)
            for t, dr in ((boutbc, bout_d), (gammabc, gamma_d), (betabc, beta_d)):
                src = dr[:, :]
                bc = bass.AP(tensor=src.tensor, offset=src.offset, ap=[[0, 128], src.ap[1]])
                nc.sync.dma_start(out=t[:], in_=bc)

            # ---------- QKV projection ----------
            # Q_T/K_T: [m, s], m-chunks: 0: Q h0-1, 1: Q h2-3, 2: K h0-1, 3: K h2-3
            qk_sb = persist.tile([128, 4, S], bf16)
            for mc in range(4):
                for sc in range(4):
                    ps = psA.tile([128, 512], f32, tag="aux")
                    for kt in range(8):
                        nc.tensor.matmul(
                            ps[:],
                            wqk_sb[:, kt, mc * 128 : (mc + 1) * 128],
                            embT_sb[:, kt, sc * 512 : (sc + 1) * 512],
                            start=(kt == 0),
                            stop=(kt == 7),
                        )
                    nc.scalar.activation(
                        out=qk_sb[:, mc, sc * 512 : (sc + 1) * 512],
                        in_=ps[:],
                        func=AF.Identity,
                        bias=bqk_sb[:, mc : mc + 1],
                        scale=1.0,
                    )

            # V: [s, (h, a+1)] bf16, ones column LAST per head (sumexp row trick)
            v_sb = persist.tile([128, 16, HL, 1 + A], bf16)
            nc.vector.memset(v_sb, 1.0)
            for st in range(16):
                ps = psA.tile([128, HL * A], f32, tag="aux")
                for kt in range(8):
                    nc.tensor.matmul(
                        ps[:],
                        embT_sb[:, kt, st * 128 : (st + 1) * 128],
                        wv_sb[:, kt, :],
                        start=(kt == 0),
                        stop=False,
                    )
                nc.tensor.matmul(ps[:], ones_r[:, :], bv_sb[:, :], start=False, stop=True)
                nc.vector.tensor_copy(
                    out=v_sb[:, st, :, 0:A],
                    in_=ps.rearrange("p (h a) -> p h a", h=HL),
                )

            # ---------- mask (reuses the embT slot; waits for last embT read) ----------
            mask_sb = big.tile([128, 16, S], bf16, tag="bigslot")
            for kb in range(16):
                nc.sync.dma_start(out=mask_sb[:, kb, :], in_=maskT_d[kb * 128 : (kb + 1) * 128, :])

            # xT rows 0..63 = a-dim of head h
            xT_sb = persist.tile([64, HL, S], bf16)
            rs_out = [
                dram.tile([128, D], bf16, name=f"rsout{q}", tag=f"rsout{q}")
                for q in range(4)
            ]

            # ---------- attention: units (q-quarter, head); tail per quarter ----------
            for quarter in range(4):
                qoff = quarter * 512
                for h in range(4):
                    kslc = slice(64 * (h % 2), 64 * (h % 2) + 64)
                    kmc = 2 + h // 2
                    qmc = h // 2
                    probs = probsp.tile([128, 16, 512], bf16, tag="probs")
                    for j in range(8):  # kb pairs
                        ps_s = psS.tile([128, 2, 512], f32, tag="score")
                        for kk in range(2):
                            kb = 2 * j + kk
                            nc.tensor.matmul(
                                ps_s[:, kk, :],
                                qk_sb[kslc, kmc, kb * 128 : (kb + 1) * 128],
                                qk_sb[kslc, qmc, qoff : qoff + 512],
                                start=True,
                                stop=True,
                            )
                        nc.scalar.activation(
                            out=probs[:, 2 * j : 2 * j + 2, :],
                            in_=ps_s[:, :, :],
                            func=AF.Exp,
                            scale=0.125,
                        )
                    for j in range(4):
                        nc.vector.tensor_tensor(
                            probs[:, 4 * j : 4 * j + 4, :],
                            probs[:, 4 * j : 4 * j + 4, :],
                            mask_sb[:, 4 * j : 4 * j + 4, qoff : qoff + 512],
                            OP.mult,
                        )
                    # PV: xT[1+a, q] = [ones|V_h].T @ probs, accumulated over kb
                    ps_x = psB.tile([65, 512], f32, tag="pvx")
                    for kb in range(16):
                        nc.tensor.matmul(
                            ps_x[:],
                            v_sb[:, kb, h, :],
                            probs[:, kb, :],
                            start=(kb == 0),
                            stop=(kb == 15),
                        )
                    # normalize: row 64 = sumexp; broadcast 1/sumexp to rows 0..64
                    recip = work.tile([65, 512], f32r, tag="recip")
                    with nc.allow_low_precision(reason="f32r is bitwise f32"):
                        nc.vector.reciprocal(recip[64:65, :], ps_x[64:65, :])
                    ps_r = psA.tile([64, 512], f32, tag="aux")
                    nc.tensor.matmul(
                        ps_r[:], ones64[64:65, 0:64], recip[64:65, :], start=True, stop=True
                    )
                    rb_sb = work.tile([64, 512], f32, tag="rbsb")
                    nc.vector.tensor_copy(out=rb_sb[:], in_=ps_r[:])
                    nc.vector.tensor_tensor(
                        xT_sb[:, h, qoff : qoff + 512],
                        ps_x[0:64, :],
                        rb_sb[:, :],
                        OP.mult,
                    )

                # ----- out-projection for this quarter + ReduceScatter + LN -----
                ar_in = dram.tile([QB, D], bf16, name=f"arin{quarter}", tag=f"arin{quarter}")
                for qc in range(4):
                    for dc in range(2):
                        ps_o = psA.tile([128, 512], f32, tag="aux")
                        for h in range(4):
                            nc.tensor.matmul(
                                ps_o[:],
                                xT_sb[:, h, qoff + qc * 128 : qoff + (qc + 1) * 128],
                                wout_sb[:, h, dc * 512 : (dc + 1) * 512],
                                start=(h == 0),
                                stop=(h == 3),
                            )
                        oe = work.tile([128, 512], bf16, tag="oevict", bufs=3)
                        nc.any.tensor_copy(out=oe[:], in_=ps_o[:])
                        nc.sync.dma_start(
                            out=ar_in[qc * 128 : (qc + 1) * 128, dc * 512 : (dc + 1) * 512],
                            in_=oe[:],
                        )
                nc.gpsimd.collective_compute(
                    "ReduceScatter",
                    OP.add,
                    replica_groups=GROUPS,
                    ins=[ar_in[:, :].opt()],
                    outs=[rs_out[quarter][:, :].opt()],
                )

                # ----- residual + LN on my 128 rows of this quarter -----
                rsl = slice(quarter * 128, (quarter + 1) * 128)
                rsb = work.tile([128, D], bf16, tag="rsb")
                nc.sync.dma_start(out=rsb[:], in_=rs_out[quarter][:, :])
                y = work.tile([128, D], f32, tag="y", bufs=1)
                er = work.tile([128, D], f32, tag="er", bufs=1)
                nc.sync.dma_start(out=er[:], in_=embres_d[rsl, :])
                nc.vector.tensor_tensor(y[:], er[:], rsb[:], OP.add)
                nc.vector.tensor_tensor(y[:], y[:], boutbc[:], OP.add)
                stats = work.tile([128, 2, nc.vector.BN_STATS_DIM], f32, tag="stats")
                for sg in range(2):
                    nc.vector.bn_stats(out=stats[:, sg, :], in_=y[:, sg * 512 : (sg + 1) * 512])
                mv = work.tile([128, nc.vector.BN_AGGR_DIM], f32, tag="mv")
                nc.vector.bn_aggr(out=mv[:], in_=stats[:])
                rstd = work.tile([128, 1], f32, tag="rstd")
                nc.scalar.activation(
                    out=rstd[:], in_=mv[:, 1:2], func=AF.Sqrt, bias=eps_sb[:], scale=1.0
                )
                nc.vector.reciprocal(rstd[:], rstd[:])
                nc.vector.tensor_scalar(
                    y[:], y[:], mv[:, 0:1], rstd[:], OP.subtract, OP.mult
                )
                o = work.tile([128, D], f32, tag="er", bufs=1)
                nc.vector.tensor_tensor(o[:], y[:], gammabc[:], OP.mult)
                nc.vector.tensor_tensor(o[:], o[:], betabc[:], OP.add)
                nc.sync.dma_start(out=out_d[rsl, :], in_=o[:])

    nc.compile()
    return nc


def _prep_inputs(embeddings, attention_mask, W_qkv, b_qkv, W_out, b_out, ln_gamma, ln_beta):
    emb = np.asarray(embeddings, dtype=np.float32)
    mask = np.asarray(attention_mask)
    W_qkv = np.asarray(W_qkv, dtype=np.float32)
    b_qkv = np.asarray(b_qkv, dtype=np.float32)
    W_out = np.asarray(W_out, dtype=np.float32)
    b_out = np.asarray(b_out, dtype=np.float32)
    gamma = np.asarray(ln_gamma, dtype=np.float32).reshape(1, D)
    beta = np.asarray(ln_beta, dtype=np.float32).reshape(1, D)

    in_maps = []
    for c in range(NCORES):
        b = c // G
        g = c % G
        hs = g * HL * A  # 256g
        embT = np.ascontiguousarray(emb[b].T)
        maskT = np.ascontiguousarray(mask[b].T).astype(ml_dtypes.bfloat16)
        wqk = np.ascontiguousarray(
            np.concatenate([W_qkv[:, hs : hs + 256], W_qkv[:, D + hs : D + hs + 256]], axis=1)
        )
        wv = np.ascontiguousarray(W_qkv[:, 2 * D + hs : 2 * D + hs + 256])
        bqk = np.concatenate([b_qkv[hs : hs + 256], b_qkv[D + hs : D + hs + 256]])
        bqk = np.ascontiguousarray(bqk.reshape(4, 128).T)
        bv = np.ascontiguousarray(b_qkv[2 * D + hs : 2 * D + hs + 256].reshape(1, 256))
        # W_out rows 256g..256g+256 as [a, h, D]
        wout = np.ascontiguousarray(
            W_out[hs : hs + 256, :].reshape(HL, A, D).transpose(1, 0, 2)
        ).astype(ml_dtypes.bfloat16)
        # my rows: for each quarter q, rows 512q + 128g .. +128
        embres = np.concatenate(
            [emb[b, 512 * q + 128 * g : 512 * q + 128 * g + 128, :] for q in range(4)],
            axis=0,
        )
        in_maps.append(
            {
                "embT": embT,
                "embres": np.ascontiguousarray(embres),
                "maskT": maskT,
                "wqk": wqk,
                "wv": wv,
                "bqk": bqk,
                "bv": bv,
                "ones": np.ones((1, 128), dtype=np.float32),
                "wout": wout,
                "bout": b_out.reshape(1, D),
                "gamma": gamma,
                "beta": beta,
            }
        )
    return in_maps


def _run(inputs, trace=False, **kw):
    if "nc" not in _CACHE:
        _CACHE["nc"] = _build()
    nc = _CACHE["nc"]
    in_maps = _prep_inputs(**inputs)
    res = run_bass_kernel_spmd(nc, in_maps, list(range(NCORES)), trace=trace, **kw)
    out = np.empty((B, S, D), dtype=np.float32)
    for c in range(NCORES):
        b, g = c // G, c % G
        for q in range(4):
            out[b, 512 * q + 128 * g : 512 * q + 128 * g + 128, :] = res.results[c][
                "out"
            ][128 * q : 128 * (q + 1), :]
    return out, res


def kernel(**inputs):
    out, _ = _run(inputs, trace=False)
    return out


# revision 25
# speedup vs baseline: 1.1268x; 1.0300x over previous
"""Multi-head self-attention block (B=2, S=2048, D=1024, H=16) on 8 TRN2 cores.

Sharding: 2-way data-parallel over batch x 4-way tensor-parallel over heads.
Core c handles batch b=c//4 with group rank g=c%4 (heads 4g..4g+4). The
out-projection partials are combined with one bf16 ReduceScatter per
q-quarter over the 4-core batch group, so core g owns output rows
[512q + 128g, 512q + 128(g+1)) for q in 0..4 — collectives overlap the
remaining attention quarters instead of forming a serial tail.

Self-contained: hardcodes all shapes; builds the Bass program once.
"""

import os
import sys

sys.path.insert(0, "/opt/trn_rl_repo")

import numpy as np
import ml_dtypes

import concourse.bass as bass
import concourse.tile as tile
from concourse import bacc, mybir
from concourse.bass_utils import run_bass_kernel_spmd

B, S, D, H = 2, 2048, 1024, 16
A = D // H  # 64
NCORES = 8
G = 4  # cores per batch group
HL = H // G  # local heads per core = 4
M_QK = 2 * HL * A  # 512 rows of Q_T+K_T per core
QB = S // G  # 512
EPS = 1e-3
GROUPS = [[0, 1, 2, 3], [4, 5, 6, 7]]

f32 = mybir.dt.float32
f32r = mybir.dt.float32r
bf16 = mybir.dt.bfloat16

AF = mybir.ActivationFunctionType
OP = mybir.AluOpType

_CACHE = {}


def _build():
    nc = bacc.Bacc("TRN2", target_bir_lowering=False, debug=False, num_devices=NCORES)

    # ---- I/O ----
    embT_d = nc.dram_tensor("embT", [D, S], f32r, kind="ExternalInput")
    embres_d = nc.dram_tensor("embres", [QB, D], f32, kind="ExternalInput")
    maskT_d = nc.dram_tensor("maskT", [S, S], bf16, kind="ExternalInput")
    wqk_d = nc.dram_tensor("wqk", [D, M_QK], f32r, kind="ExternalInput")
    wv_d = nc.dram_tensor("wv", [D, HL * A], f32r, kind="ExternalInput")
    bqk_d = nc.dram_tensor("bqk", [128, 4], f32, kind="ExternalInput")
    bv_d = nc.dram_tensor("bv", [1, HL * A], f32r, kind="ExternalInput")
    ones_d = nc.dram_tensor("ones", [1, 128], f32r, kind="ExternalInput")
    # W_out shard as [a=64, local head, D]
    wout_d = nc.dram_tensor("wout", [64, HL, D], bf16, kind="ExternalInput")
    bout_d = nc.dram_tensor("bout", [1, D], f32, kind="ExternalInput")
    gamma_d = nc.dram_tensor("gamma", [1, D], f32, kind="ExternalInput")
    beta_d = nc.dram_tensor("beta", [1, D], f32, kind="ExternalInput")
    out_d = nc.dram_tensor("out", [QB, D], f32, kind="ExternalOutput")

    with tile.TileContext(nc) as tc:
        with (
            tc.tile_pool(name="big", bufs=1) as big,  # embT then maskT (64KB/p slot)
            tc.tile_pool(name="persist", bufs=1) as persist,
            tc.tile_pool(name="probs", bufs=2) as probsp,
            tc.tile_pool(name="work", bufs=2) as work,
            tc.tile_pool(name="psA", bufs=2, space="PSUM") as psA,  # 1-bank f32 mm
            tc.tile_pool(name="psS", bufs=2, space="PSUM") as psS,  # scores (2 banks)
            tc.tile_pool(name="psB", bufs=2, space="PSUM") as psB,  # pv xT (1 bank)
            tc.tile_pool(name="dram", bufs=1, space="DRAM") as dram,
        ):
            # ---------- embT first: it gates the QKV critical path ----------
            embT_sb = big.tile([128, 8, S], f32r, tag="bigslot")
            for kt in range(8):
                nc.sync.dma_start(out=embT_sb[:, kt, :], in_=embT_d[kt * 128 : (kt + 1) * 128, :])

            # ---------- weights / constants ----------
            wqk_sb = persist.tile([128, 8, M_QK], f32r)
            wv_sb = persist.tile([128, 8, HL * A], f32r)
            for kt in range(8):
                nc.sync.dma_start(out=wqk_sb[:, kt, :], in_=wqk_d[kt * 128 : (kt + 1) * 128, :])
                nc.sync.dma_start(out=wv_sb[:, kt, :], in_=wv_d[kt * 128 : (kt + 1) * 128, :])
            bqk_sb = persist.tile([128, 4], f32)
            nc.sync.dma_start(out=bqk_sb, in_=bqk_d[:, :])
            bv_sb = persist.tile([1, HL * A], f32r)
            nc.sync.dma_start(out=bv_sb, in_=bv_d[:, :])
            ones_r = persist.tile([1, 128], f32r)
            nc.sync.dma_start(out=ones_r, in_=ones_d[:, :])
            ones64 = persist.tile([65, 128], f32r)
            nc.sync.dma_start(out=ones64[64:65, :], in_=ones_d[:, :])
            wout_sb = persist.tile([64, HL, D], bf16)
            nc.sync.dma_start(out=wout_sb, in_=wout_d[:, :, :])
            eps_sb = persist.tile([128, 1], f32)
            nc.vector.memset(eps_sb, EPS)
            boutbc = persist.tile([128, D], f32)
            gammabc = persist.tile([128, D], f32)
            betabc = persist.tile([128, D], f32)
            for t, dr in ((boutbc, bout_d), (gammabc, gamma_d), (betabc, beta_d)):
                src = dr[:, :]
                bc = bass.AP(tensor=src.tensor, offset=src.offset, ap=[[0, 128], src.ap[1]])
                nc.sync.dma_start(out=t[:], in_=bc)

            # ---------- QKV projection ----------
            # Q_T/K_T: [m, s], m-chunks: 0: Q h0-1, 1: Q h2-3, 2: K h0-1, 3: K h2-3
            qk_sb = persist.tile([128, 4, S], bf16)
            for mc in range(4):
                for sc in range(4):
                    ps = psA.tile([128, 512], f32, tag="aux")
                    for kt in range(8):
                        nc.tensor.matmul(
                            ps[:],
                            wqk_sb[:, kt, mc * 128 : (mc + 1) * 128],
                            embT_sb[:, kt, sc * 512 : (sc + 1) * 512],
                            start=(kt == 0),
                            stop=(kt == 7),
                        )
                    nc.scalar.activation(
                        out=qk_sb[:, mc, sc * 512 : (sc + 1) * 512],
                        in_=ps[:],
                        func=AF.Identity,
                        bias=bqk_sb[:, mc : mc + 1],
                        scale=1.0,
                    )

            # V: [s, (h, a+1)] bf16, ones column LAST per head (sumexp row trick)
            v_sb = persist.tile([128, 16, HL, 1 + A], bf16)
            nc.vector.memset(v_sb, 1.0)
            for st in range(16):
                ps = psA.tile([128, HL * A], f32, tag="aux")
                for kt in range(8):
                    nc.tensor.matmul(
                        ps[:],
                        embT_sb[:, kt, st * 128 : (st + 1) * 128],
                        wv_sb[:, kt, :],
                        start=(kt == 0),
                        stop=False,
                    )
                nc.tensor.matmul(ps[:], ones_r[:, :], bv_sb[:, :], start=False, stop=True)
                nc.vector.tensor_copy(
                    out=v_sb[:, st, :, 0:A],
                    in_=ps.rearrange("p (h a) -> p h a", h=HL),
                )

            # ---------- mask (reuses the embT slot; waits for last embT read) ----------
            mask_sb = big.tile([128, 16, S], bf16, tag="bigslot")
            for kb in range(16):
                nc.sync.dma_start(out=mask_sb[:, kb, :], in_=maskT_d[kb * 128 : (kb + 1) * 128, :])

            # xT rows 0..63 = a-dim of head h
            xT_sb = persist.tile([64, HL, S], bf16)
            rs_out = [
                dram.tile([128, D], bf16, name=f"rsout{q}", tag=f"rsout{q}")
                for q in range(4)
            ]

            # ---------- attention, software-pipelined by one unit ----------
            # Unit u = (quarter, head): scores+exp+mask of u are interleaved
            # with the PV matmuls of u-1 (fills PE gaps while ACT runs exp);
            # the per-quarter tail (out-proj + ReduceScatter + LN) is emitted
            # one unit late so collectives overlap the next quarter.

            def pv_mms(pu, kb0, kb1):
                (pq, ph, pprobs, pps_x) = pu
                for kb in range(kb0, kb1):
                    nc.tensor.matmul(
                        pps_x[:],
                        v_sb[:, kb, ph, :],
                        pprobs[:, kb, :],
                        start=(kb == 0),
                        stop=(kb == 15),
                    )

            def normalize_evict(pu):
                (pq, ph, pprobs, pps_x) = pu
                # row 64 = sumexp; broadcast 1/sumexp via PE, then scale rows 0..64
                recip = work.tile([65, 512], f32r, tag="recip")
                with nc.allow_low_precision(reason="f32r is bitwise f32"):
                    nc.vector.reciprocal(recip[64:65, :], pps_x[64:65, :])
                ps_r = psA.tile([64, 512], f32, tag="aux")
                nc.tensor.matmul(
                    ps_r[:], ones64[64:65, 0:64], recip[64:65, :], start=True, stop=True
                )
                rb_sb = work.tile([64, 512], f32, tag="rbsb")
                nc.vector.tensor_copy(out=rb_sb[:], in_=ps_r[:])
                nc.vector.tensor_tensor(
                    xT_sb[:, ph, pq * 512 : pq * 512 + 512],
                    pps_x[0:64, :],
                    rb_sb[:, :],
                    OP.mult,
                )

            def quarter_tail(q):
                qo = q * 512
                ar_in = dram.tile([QB, D], bf16, name=f"arin{q}", tag=f"arin{q}")
                for qc in range(4):
                    for dc in range(2):
                        ps_o = psA.tile([128, 512], f32, tag="aux")
                        for h in range(4):
                            nc.tensor.matmul(
                                ps_o[:],
                                xT_sb[:, h, qo + qc * 128 : qo + (qc + 1) * 128],
                                wout_sb[:, h, dc * 512 : (dc + 1) * 512],
                                start=(h == 0),
                                stop=(h == 3),
                            )
                        oe = work.tile([128, 512], bf16, tag="oevict", bufs=3)
                        nc.any.tensor_copy(out=oe[:], in_=ps_o[:])
                        nc.sync.dma_start(
                            out=ar_in[qc * 128 : (qc + 1) * 128, dc * 512 : (dc + 1) * 512],
                            in_=oe[:],
                        )
                nc.gpsimd.collective_compute(
                    "ReduceScatter",
                    OP.add,
                    replica_groups=GROUPS,
                    ins=[ar_in[:, :].opt()],
                    outs=[rs_out[q][:, :].opt()],
                )
                # residual + LN on my 128 rows of this quarter
                rsl = slice(q * 128, (q + 1) * 128)
                rsb = work.tile([128, D], bf16, tag="rsb")
                nc.sync.dma_start(out=rsb[:], in_=rs_out[q][:, :])
                y = work.tile([128, D], f32, tag="y", bufs=1)
                er = work.tile([128, D], f32, tag="er", bufs=1)
                nc.sync.dma_start(out=er[:], in_=embres_d[rsl, :])
                nc.vector.tensor_tensor(y[:], er[:], rsb[:], OP.add)
                nc.vector.tensor_tensor(y[:], y[:], boutbc[:], OP.add)
                stats = work.tile([128, 2, nc.vector.BN_STATS_DIM], f32, tag="stats")
                for sg in range(2):
                    nc.vector.bn_stats(out=stats[:, sg, :], in_=y[:, sg * 512 : (sg + 1) * 512])
                mv = work.tile([128, nc.vector.BN_AGGR_DIM], f32, tag="mv")
                nc.vector.bn_aggr(out=mv[:], in_=stats[:])
                rstd = work.tile([128, 1], f32, tag="rstd")
                nc.scalar.activation(
                    out=rstd[:], in_=mv[:, 1:2], func=AF.Sqrt, bias=eps_sb[:], scale=1.0
                )
                nc.vector.reciprocal(rstd[:], rstd[:])
                nc.vector.tensor_scalar(
                    y[:], y[:], mv[:, 0:1], rstd[:], OP.subtract, OP.mult
                )
                o = work.tile([128, D], f32, tag="er", bufs=1)
                nc.vector.tensor_tensor(o[:], y[:], gammabc[:], OP.mult)
                nc.vector.tensor_tensor(o[:], o[:], betabc[:], OP.add)
                nc.sync.dma_start(out=out_d[rsl, :], in_=o[:])

            prev = None
            for quarter in range(4):
                qoff = quarter * 512
                for h in range(4):
                    kslc = slice(64 * (h % 2), 64 * (h % 2) + 64)
                    kmc = 2 + h // 2
                    qmc = h // 2
                    probs = probsp.tile([128, 16, 512], bf16, tag="probs")
                    ps_x = psB.tile([65, 512], f32, tag="pvx")
                    for j in range(8):  # kb pairs
                        ps_s = psS.tile([128, 2, 512], f32, tag="score")
                        for kk in range(2):
                            kb = 2 * j + kk
                            nc.tensor.matmul(
                                ps_s[:, kk, :],
                                qk_sb[kslc, kmc, kb * 128 : (kb + 1) * 128],
                                qk_sb[kslc, qmc, qoff : qoff + 512],
                                start=True,
                                stop=True,
                            )
                        if prev is not None:
                            pv_mms(prev, 2 * j, 2 * j + 2)
                        nc.scalar.activation(
                            out=probs[:, 2 * j : 2 * j + 2, :],
                            in_=ps_s[:, :, :],
                            func=AF.Exp,
                            scale=0.125,
                        )
                        nc.vector.tensor_tensor(
                            probs[:, 2 * j : 2 * j + 2, :],
                            probs[:, 2 * j : 2 * j + 2, :],
                            mask_sb[:, 2 * j : 2 * j + 2, qoff : qoff + 512],
                            OP.mult,
                        )
                    if prev is not None:
                        normalize_evict(prev)
                        if prev[1] == 3:  # prev closed quarter prev[0]
                            quarter_tail(prev[0])
                    prev = (quarter, h, probs, ps_x)
            pv_mms(prev, 0, 16)
            normalize_evict(prev)
            quarter_tail(3)

    nc.compile()
    return nc


def _prep_inputs(embeddings, attention_mask, W_qkv, b_qkv, W_out, b_out, ln_gamma, ln_beta):
    emb = np.asarray(embeddings, dtype=np.float32)
    mask = np.asarray(attention_mask)
    W_qkv = np.asarray(W_qkv, dtype=np.float32)
    b_qkv = np.asarray(b_qkv, dtype=np.float32)
    W_out = np.asarray(W_out, dtype=np.float32)
    b_out = np.asarray(b_out, dtype=np.float32)
    gamma = np.asarray(ln_gamma, dtype=np.float32).reshape(1, D)
    beta = np.asarray(ln_beta, dtype=np.float32).reshape(1, D)

    in_maps = []
    for c in range(NCORES):
        b = c // G
        g = c % G
        hs = g * HL * A  # 256g
        embT = np.ascontiguousarray(emb[b].T)
        maskT = np.ascontiguousarray(mask[b].T).astype(ml_dtypes.bfloat16)
        wqk = np.ascontiguousarray(
            np.concatenate([W_qkv[:, hs : hs + 256], W_qkv[:, D + hs : D + hs + 256]], axis=1)
        )
        wv = np.ascontiguousarray(W_qkv[:, 2 * D + hs : 2 * D + hs + 256])
        bqk = np.concatenate([b_qkv[hs : hs + 256], b_qkv[D + hs : D + hs + 256]])
        bqk = np.ascontiguousarray(bqk.reshape(4, 128).T)
        bv = np.ascontiguousarray(b_qkv[2 * D + hs : 2 * D + hs + 256].reshape(1, 256))
        # W_out rows 256g..256g+256 as [a, h, D]
        wout = np.ascontiguousarray(
            W_out[hs : hs + 256, :].reshape(HL, A, D).transpose(1, 0, 2)
        ).astype(ml_dtypes.bfloat16)
        # my rows: for each quarter q, rows 512q + 128g .. +128
        embres = np.concatenate(
            [emb[b, 512 * q + 128 * g : 512 * q + 128 * g + 128, :] for q in range(4)],
            axis=0,
        )
        in_maps.append(
            {
                "embT": embT,
                "embres": np.ascontiguousarray(embres),
                "maskT": maskT,
                "wqk": wqk,
                "wv": wv,
                "bqk": bqk,
                "bv": bv,
                "ones": np.ones((1, 128), dtype=np.float32),
                "wout": wout,
                "bout": b_out.reshape(1, D),
                "gamma": gamma,
                "beta": beta,
            }
        )
    return in_maps


def _run(inputs, trace=False, **kw):
    if "nc" not in _CACHE:
        _CACHE["nc"] = _build()
    nc = _CACHE["nc"]
    in_maps = _prep_inputs(**inputs)
    res = run_bass_kernel_spmd(nc, in_maps, list(range(NCORES)), trace=trace, **kw)
    out = np.empty((B, S, D), dtype=np.float32)
    for c in range(NCORES):
        b, g = c // G, c % G
        for q in range(4):
            out[b, 512 * q + 128 * g : 512 * q + 128 * g + 128, :] = res.results[c][
                "out"
            ][128 * q : 128 * (q + 1), :]
    return out, res


def kernel(**inputs):
    out, _ = _run(inputs, trace=False)
    return out


# revision 27
# speedup vs baseline: 1.3416x; 1.1906x over previous
"""Multi-head self-attention block (B=2, S=2048, D=1024, H=16) on 8 TRN2 cores.

Sharding: 2-way data-parallel over batch x 4-way tensor-parallel over heads.
Core c handles batch b=c//4 with group rank g=c%4 (heads 4g..4g+4). The
out-projection partials are combined with one bf16 ReduceScatter per
q-quarter over the 4-core batch group, so core g owns output rows
[512q + 128g, 512q + 128(g+1)) for q in 0..4 — collectives overlap the
remaining attention quarters instead of forming a serial tail.

Score matmuls are packed block-diagonally (two 64-wide k-halves on the
128 partitions with Q duplicated) so the PE array runs fully active —
half-array matmuls keep the HAM clock gate at 1.2 GHz.

Self-contained: hardcodes all shapes; builds the Bass program once.
"""

import os
import sys

sys.path.insert(0, "/opt/trn_rl_repo")

import numpy as np
import ml_dtypes

import concourse.bass as bass
import concourse.tile as tile
from concourse import bacc, mybir
from concourse.bass_utils import run_bass_kernel_spmd

B, S, D, H = 2, 2048, 1024, 16
A = D // H  # 64
NCORES = 8
G = 4  # cores per batch group
HL = H // G  # local heads per core = 4
M_QK = 2 * HL * A  # 512 rows of Q_T+K_T per core
QB = S // G  # 512
EPS = 1e-3
GROUPS = [[0, 1, 2, 3], [4, 5, 6, 7]]

f32 = mybir.dt.float32
f32r = mybir.dt.float32r
bf16 = mybir.dt.bfloat16

AF = mybir.ActivationFunctionType
OP = mybir.AluOpType

_CACHE = {}


def _build():
    nc = bacc.Bacc("TRN2", target_bir_lowering=False, debug=False, num_devices=NCORES)

    # ---- I/O ----
    embT_d = nc.dram_tensor("embT", [D, S], bf16, kind="ExternalInput")
    embres_d = nc.dram_tensor("embres", [QB, D], f32, kind="ExternalInput")
    maskT_d = nc.dram_tensor("maskT", [S, S], bf16, kind="ExternalInput")
    wqk_d = nc.dram_tensor("wqk", [D, M_QK], bf16, kind="ExternalInput")
    wv_d = nc.dram_tensor("wv", [D, HL * A], bf16, kind="ExternalInput")
    bqk_d = nc.dram_tensor("bqk", [128, 4], f32, kind="ExternalInput")
    bv_d = nc.dram_tensor("bv", [1, HL * A], bf16, kind="ExternalInput")
    ones_d = nc.dram_tensor("ones", [1, 128], f32r, kind="ExternalInput")
    onesb_d = nc.dram_tensor("onesb", [1, 128], bf16, kind="ExternalInput")
    wout_d = nc.dram_tensor("wout", [128, 2, D], bf16, kind="ExternalInput")
    bout_d = nc.dram_tensor("bout", [1, D], f32, kind="ExternalInput")
    gamma_d = nc.dram_tensor("gamma", [1, D], f32, kind="ExternalInput")
    beta_d = nc.dram_tensor("beta", [1, D], f32, kind="ExternalInput")
    out_d = nc.dram_tensor("out", [QB, D], f32, kind="ExternalOutput")

    with tile.TileContext(nc) as tc:
        with (
            tc.tile_pool(name="big", bufs=1) as big,  # embT then maskT (64KB/p slot)
            tc.tile_pool(name="persist", bufs=1) as persist,
            tc.tile_pool(name="probs", bufs=2) as probsp,
            tc.tile_pool(name="work", bufs=2) as work,
            tc.tile_pool(name="psA", bufs=2, space="PSUM") as psA,  # 1-bank f32 mm
            tc.tile_pool(name="psS", bufs=2, space="PSUM") as psS,  # scores (2 banks)
            tc.tile_pool(name="psB", bufs=2, space="PSUM") as psB,  # pv xT (1 bank)
            tc.tile_pool(name="dram", bufs=1, space="DRAM") as dram,
        ):
            # ---------- embT first: it gates the QKV critical path ----------
            embT_sb = big.tile([128, 8, S], bf16, tag="bigslot", padded_shape=[128, 16, S])
            for kt in range(8):
                nc.sync.dma_start(out=embT_sb[:, kt, :], in_=embT_d[kt * 128 : (kt + 1) * 128, :])

            # ---------- weights / constants ----------
            wqk_sb = persist.tile([128, 8, M_QK], bf16)
            wv_sb = persist.tile([128, 8, HL * A], bf16)
            for kt in range(8):
                nc.sync.dma_start(out=wqk_sb[:, kt, :], in_=wqk_d[kt * 128 : (kt + 1) * 128, :])
                nc.sync.dma_start(out=wv_sb[:, kt, :], in_=wv_d[kt * 128 : (kt + 1) * 128, :])
            bqk_sb = persist.tile([128, 4], f32)
            nc.sync.dma_start(out=bqk_sb, in_=bqk_d[:, :])
            bv_sb = persist.tile([1, HL * A], bf16)
            nc.sync.dma_start(out=bv_sb, in_=bv_d[:, :])
            ones_b = persist.tile([1, 128], bf16)
            nc.sync.dma_start(out=ones_b, in_=onesb_d[:, :])
            ones64 = persist.tile([65, 128], f32r)
            nc.sync.dma_start(out=ones64[64:65, :], in_=ones_d[:, :])
            wout_sb = persist.tile([128, 2, D], bf16)
            nc.sync.dma_start(out=wout_sb, in_=wout_d[:, :, :])
            eps_sb = persist.tile([128, 1], f32)
            nc.vector.memset(eps_sb, EPS)
            boutbc = persist.tile([128, D], f32)
            gammabc = persist.tile([128, D], f32)
            betabc = persist.tile([128, D], f32)
            for t, dr in ((boutbc, bout_d), (gammabc, gamma_d), (betabc, beta_d)):
                src = dr[:, :]
                bc = bass.AP(tensor=src.tensor, offset=src.offset, ap=[[0, 128], src.ap[1]])
                nc.sync.dma_start(out=t[:], in_=bc)

            # ---------- QKV projection ----------
            # Q duplicated on both partition halves: q2[p, h, s], p<64 and
            # p>=64 both hold Q_h[p % 64, s].
            q2_sb = persist.tile([128, HL, S], bf16)
            # K block-diag: k2[0:64, h, kb, 0:64] = K_h[a, 128 kb + m],
            # k2[64:128, h, kb, 64:128] = K_h[a, 128 kb + 64 + m], zeros off-diag.
            k2_sb = persist.tile([128, HL, 16, 128], bf16)
            nc.vector.memset(k2_sb, 0.0)

            for mc in range(4):  # 0: Q h0-1, 1: Q h2-3, 2: K h0-1, 3: K h2-3
                for sc in range(4):
                    ps = psA.tile([128, 512], f32, tag="aux")
                    for kt in range(8):
                        nc.tensor.matmul(
                            ps[:],
                            wqk_sb[:, kt, mc * 128 : (mc + 1) * 128],
                            embT_sb[:, kt, sc * 512 : (sc + 1) * 512],
                            start=(kt == 0),
                            stop=(kt == 7),
                        )
                    if mc < 2:  # Q: natural-half eviction per head
                        he, ho = 2 * mc, 2 * mc + 1
                        nc.scalar.activation(
                            out=q2_sb[0:64, he, sc * 512 : (sc + 1) * 512],
                            in_=ps[0:64, :],
                            func=AF.Identity,
                            bias=bqk_sb[0:64, mc : mc + 1],
                            scale=1.0,
                        )
                        nc.scalar.activation(
                            out=q2_sb[64:128, ho, sc * 512 : (sc + 1) * 512],
                            in_=ps[64:128, :],
                            func=AF.Identity,
                            bias=bqk_sb[64:128, mc : mc + 1],
                            scale=1.0,
                        )
                    else:  # K: stage, then scatter into the diag blocks via DMA
                        he, ho = 2 * (mc - 2), 2 * (mc - 2) + 1
                        kstage = work.tile([128, 512], bf16, tag="kstage")
                        nc.scalar.activation(
                            out=kstage[:],
                            in_=ps[:],
                            func=AF.Identity,
                            bias=bqk_sb[:, mc : mc + 1],
                            scale=1.0,
                        )
                        ks = kstage.rearrange("p (k t m) -> p k t m", k=4, m=64)
                        kslc = slice(sc * 4, (sc + 1) * 4)
                        nc.sync.dma_start(
                            out=k2_sb[0:64, he, kslc, 0:64], in_=ks[0:64, :, 0, :]
                        )
                        nc.sync.dma_start(
                            out=k2_sb[64:128, he, kslc, 64:128], in_=ks[0:64, :, 1, :]
                        )
                        nc.sync.dma_start(
                            out=k2_sb[0:64, ho, kslc, 0:64], in_=ks[64:128, :, 0, :]
                        )
                        nc.sync.dma_start(
                            out=k2_sb[64:128, ho, kslc, 64:128], in_=ks[64:128, :, 1, :]
                        )
            # duplicate Q across partition halves (SBUF->SBUF DMA moves partitions)
            for h in range(HL):
                if h % 2 == 0:
                    nc.sync.dma_start(out=q2_sb[64:128, h, :], in_=q2_sb[0:64, h, :])
                else:
                    nc.sync.dma_start(out=q2_sb[0:64, h, :], in_=q2_sb[64:128, h, :])

            # V: [s, (h, a+1)] bf16, ones column LAST per head (sumexp row trick)
            v_sb = persist.tile([128, 16, HL, 1 + A], bf16)
            nc.vector.memset(v_sb, 1.0)
            for st in range(16):
                ps = psA.tile([128, HL * A], f32, tag="aux")
                for kt in range(8):
                    nc.tensor.matmul(
                        ps[:],
                        embT_sb[:, kt, st * 128 : (st + 1) * 128],
                        wv_sb[:, kt, :],
                        start=(kt == 0),
                        stop=False,
                    )
                nc.tensor.matmul(ps[:], ones_b[:, :], bv_sb[:, :], start=False, stop=True)
                nc.vector.tensor_copy(
                    out=v_sb[:, st, :, 0:A],
                    in_=ps.rearrange("p (h a) -> p h a", h=HL),
                )

            # ---------- mask (reuses the embT slot; waits for last embT read) ----------
            mask_sb = big.tile([128, 16, S], bf16, tag="bigslot")
            for kb in range(16):
                nc.sync.dma_start(out=mask_sb[:, kb, :], in_=maskT_d[kb * 128 : (kb + 1) * 128, :])

            # xT: [(a), head pair, s] — heads stacked two per 128 partitions
            xT_sb = persist.tile([128, 2, S], bf16)
            rs_out = [
                dram.tile([128, D], bf16, name=f"rsout{q}", tag=f"rsout{q}")
                for q in range(4)
            ]

            # ---------- attention, software-pipelined by one unit ----------
            def pv_mms(pu, kb0, kb1):
                (pq, ph, pprobs, pps_x) = pu
                for kb in range(kb0, kb1):
                    nc.tensor.matmul(
                        pps_x[:],
                        v_sb[:, kb, ph, :],
                        pprobs[:, kb, :],
                        start=(kb == 0),
                        stop=(kb == 15),
                    )

            def normalize_evict(pu):
                (pq, ph, pprobs, pps_x) = pu
                qo = pq * 512
                recip = work.tile([65, 512], f32r, tag="recip")
                with nc.allow_low_precision(reason="f32r is bitwise f32"):
                    nc.vector.reciprocal(recip[64:65, :], pps_x[64:65, :])
                ps_r = psA.tile([64, 512], f32, tag="aux")
                nc.tensor.matmul(
                    ps_r[:], ones64[64:65, 0:64], recip[64:65, :], start=True, stop=True
                )
                rb_sb = work.tile([64, 512], f32, tag="rbsb")
                nc.vector.tensor_copy(out=rb_sb[:], in_=ps_r[:])
                if ph % 2 == 0:
                    nc.vector.tensor_tensor(
                        xT_sb[0:64, ph // 2, qo : qo + 512],
                        pps_x[0:64, :],
                        rb_sb[:, :],
                        OP.mult,
                    )
                else:
                    xodd = work.tile([64, 512], bf16, tag="xodd")
                    nc.vector.tensor_tensor(
                        xodd[:], pps_x[0:64, :], rb_sb[:, :], OP.mult
                    )
                    nc.sync.dma_start(
                        out=xT_sb[64:128, ph // 2, qo : qo + 512], in_=xodd[:]
                    )

            def quarter_tail(q):
                qo = q * 512
                ar_in = dram.tile([QB, D], bf16, name=f"arin{q}", tag=f"arin{q}")
                for qc in range(4):
                    for dc in range(2):
                        ps_o = psA.tile([128, 512], f32, tag="aux")
                        for hp in range(2):
                            nc.tensor.matmul(
                                ps_o[:],
                                xT_sb[:, hp, qo + qc * 128 : qo + (qc + 1) * 128],
                                wout_sb[:, hp, dc * 512 : (dc + 1) * 512],
                                start=(hp == 0),
                                stop=(hp == 1),
                            )
                        oe = work.tile([128, 512], bf16, tag="oevict", bufs=3)
                        nc.any.tensor_copy(out=oe[:], in_=ps_o[:])
                        nc.sync.dma_start(
                            out=ar_in[qc * 128 : (qc + 1) * 128, dc * 512 : (dc + 1) * 512],
                            in_=oe[:],
                        )
                nc.gpsimd.collective_compute(
                    "ReduceScatter",
                    OP.add,
                    replica_groups=GROUPS,
                    ins=[ar_in[:, :].opt()],
                    outs=[rs_out[q][:, :].opt()],
                )
                # residual + LN on my 128 rows of this quarter
                rsl = slice(q * 128, (q + 1) * 128)
                rsb = work.tile([128, D], bf16, tag="rsb")
                nc.sync.dma_start(out=rsb[:], in_=rs_out[q][:, :])
                y = work.tile([128, D], f32, tag="y", bufs=1)
                er = work.tile([128, D], f32, tag="er", bufs=1)
                nc.sync.dma_start(out=er[:], in_=embres_d[rsl, :])
                nc.vector.tensor_tensor(y[:], er[:], rsb[:], OP.add)
                nc.vector.tensor_tensor(y[:], y[:], boutbc[:], OP.add)
                stats = work.tile([128, 2, nc.vector.BN_STATS_DIM], f32, tag="stats")
                for sg in range(2):
                    nc.vector.bn_stats(out=stats[:, sg, :], in_=y[:, sg * 512 : (sg + 1) * 512])
                mv = work.tile([128, nc.vector.BN_AGGR_DIM], f32, tag="mv")
                nc.vector.bn_aggr(out=mv[:], in_=stats[:])
                rstd = work.tile([128, 1], f32, tag="rstd")
                nc.scalar.activation(
                    out=rstd[:], in_=mv[:, 1:2], func=AF.Sqrt, bias=eps_sb[:], scale=1.0
                )
                nc.vector.reciprocal(rstd[:], rstd[:])
                nc.vector.tensor_scalar(
                    y[:], y[:], mv[:, 0:1], rstd[:], OP.subtract, OP.mult
                )
                o = work.tile([128, D], f32, tag="er", bufs=1)
                nc.vector.tensor_tensor(o[:], y[:], gammabc[:], OP.mult)
                nc.vector.tensor_tensor(o[:], o[:], betabc[:], OP.add)
                nc.sync.dma_start(out=out_d[rsl, :], in_=o[:])

            prev = None
            for quarter in range(4):
                qoff = quarter * 512
                for h in range(4):
                    probs = probsp.tile([128, 16, 512], bf16, tag="probs")
                    ps_x = psB.tile([65, 512], f32, tag="pvx")
                    for j in range(8):  # kb pairs
                        ps_s = psS.tile([128, 2, 512], f32, tag="score")
                        for kk in range(2):
                            kb = 2 * j + kk
                            nc.tensor.matmul(
                                ps_s[:, kk, :],
                                k2_sb[:, h, kb, :],
                                q2_sb[:, h, qoff : qoff + 512],
                                start=True,
                                stop=True,
                            )
                        if prev is not None:
                            pv_mms(prev, 2 * j, 2 * j + 2)
                        nc.scalar.activation(
                            out=probs[:, 2 * j : 2 * j + 2, :],
                            in_=ps_s[:, :, :],
                            func=AF.Exp,
                            scale=0.125,
                        )
                        nc.vector.tensor_tensor(
                            probs[:, 2 * j : 2 * j + 2, :],
                            probs[:, 2 * j : 2 * j + 2, :],
                            mask_sb[:, 2 * j : 2 * j + 2, qoff : qoff + 512],
                            OP.mult,
                        )
                    if prev is not None:
                        normalize_evict(prev)
                        if prev[1] == 3:  # prev closed quarter prev[0]
                            quarter_tail(prev[0])
                    prev = (quarter, h, probs, ps_x)
            pv_mms(prev, 0, 16)
            normalize_evict(prev)
            quarter_tail(3)

    nc.compile()
    return nc


def _prep_inputs(embeddings, attention_mask, W_qkv, b_qkv, W_out, b_out, ln_gamma, ln_beta):
    emb = np.asarray(embeddings, dtype=np.float32)
    mask = np.asarray(attention_mask)
    W_qkv = np.asarray(W_qkv, dtype=np.float32)
    b_qkv = np.asarray(b_qkv, dtype=np.float32)
    W_out = np.asarray(W_out, dtype=np.float32)
    b_out = np.asarray(b_out, dtype=np.float32)
    gamma = np.asarray(ln_gamma, dtype=np.float32).reshape(1, D)
    beta = np.asarray(ln_beta, dtype=np.float32).reshape(1, D)

    in_maps = []
    for c in range(NCORES):
        b = c // G
        g = c % G
        hs = g * HL * A  # 256g
        embT = np.ascontiguousarray(emb[b].T).astype(ml_dtypes.bfloat16)
        maskT = np.ascontiguousarray(mask[b].T).astype(ml_dtypes.bfloat16)
        wqk = np.ascontiguousarray(
            np.concatenate([W_qkv[:, hs : hs + 256], W_qkv[:, D + hs : D + hs + 256]], axis=1)
        ).astype(ml_dtypes.bfloat16)
        wv = np.ascontiguousarray(W_qkv[:, 2 * D + hs : 2 * D + hs + 256]).astype(
            ml_dtypes.bfloat16
        )
        bqk = np.concatenate([b_qkv[hs : hs + 256], b_qkv[D + hs : D + hs + 256]])
        bqk = np.ascontiguousarray(bqk.reshape(4, 128).T)
        bv = np.ascontiguousarray(
            b_qkv[2 * D + hs : 2 * D + hs + 256].reshape(1, 256)
        ).astype(ml_dtypes.bfloat16)
        wout = np.ascontiguousarray(
            W_out[hs : hs + 256, :].reshape(2, 128, D).transpose(1, 0, 2)
        ).astype(ml_dtypes.bfloat16)
        embres = np.concatenate(
            [emb[b, 512 * q + 128 * g : 512 * q + 128 * g + 128, :] for q in range(4)],
            axis=0,
        )
        in_maps.append(
            {
                "embT": embT,
                "embres": np.ascontiguousarray(embres),
                "maskT": maskT,
                "wqk": wqk,
                "wv": wv,
                "bqk": bqk,
                "bv": bv,
                "ones": np.ones((1, 128), dtype=np.float32),
                "onesb": np.ones((1, 128), dtype=ml_dtypes.bfloat16),
                "wout": wout,
                "bout": b_out.reshape(1, D),
                "gamma": gamma,
                "beta": beta,
            }
        )
    return in_maps


def _run(inputs, trace=False, **kw):
    if "nc" not in _CACHE:
        _CACHE["nc"] = _build()
    nc = _CACHE["nc"]
    in_maps = _prep_inputs(**inputs)
    res = run_bass_kernel_spmd(nc, in_maps, list(range(NCORES)), trace=trace, **kw)
    out = np.empty((B, S, D), dtype=np.float32)
    for c in range(NCORES):
        b, g = c // G, c % G
        for q in range(4):
            out[b, 512 * q + 128 * g : 512 * q + 128 * g + 128, :] = res.results[c][
                "out"
            ][128 * q : 128 * (q + 1), :]
    return out, res


def kernel(**inputs):
    out, _ = _run(inputs, trace=False)
    return out


# revision 31
# speedup vs baseline: 1.3902x; 1.0362x over previous
"""Multi-head self-attention block (B=2, S=2048, D=1024, H=16) on 8 TRN2 cores.

Sharding: 2-way data-parallel over batch x 4-way tensor-parallel over heads.
Core c handles batch b=c//4 with group rank g=c%4 (heads 4g..4g+4). The
out-projection partials are combined with one bf16 ReduceScatter per
q-quarter over the 4-core batch group, so core g owns output rows
[512q + 128g, 512q + 128(g+1)) for q in 0..4 — collectives overlap the
remaining attention quarters instead of forming a serial tail.

Score matmuls are packed block-diagonally (two 64-wide k-halves on the
128 partitions with Q duplicated) so the PE array runs fully active —
half-array matmuls keep the HAM clock gate at 1.2 GHz.

Self-contained: hardcodes all shapes; builds the Bass program once.
"""

import os
import sys

sys.path.insert(0, "/opt/trn_rl_repo")

import numpy as np
import ml_dtypes

import concourse.bass as bass
import concourse.tile as tile
from concourse import bacc, mybir
from concourse.bass_utils import run_bass_kernel_spmd

B, S, D, H = 2, 2048, 1024, 16
A = D // H  # 64
NCORES = 8
G = 4  # cores per batch group
HL = H // G  # local heads per core = 4
M_QK = 2 * HL * A  # 512 rows of Q_T+K_T per core
QB = S // G  # 512
EPS = 1e-3
GROUPS = [[0, 1, 2, 3], [4, 5, 6, 7]]

f32 = mybir.dt.float32
f32r = mybir.dt.float32r
bf16 = mybir.dt.bfloat16

AF = mybir.ActivationFunctionType
OP = mybir.AluOpType

_CACHE = {}


def _build():
    nc = bacc.Bacc("TRN2", target_bir_lowering=False, debug=False, num_devices=NCORES)

    # ---- I/O ----
    embT_d = nc.dram_tensor("embT", [D, S], bf16, kind="ExternalInput")
    embres_d = nc.dram_tensor("embres", [QB, D], f32, kind="ExternalInput")
    maskT_d = nc.dram_tensor("maskT", [S, S], bf16, kind="ExternalInput")
    wqk_d = nc.dram_tensor("wqk", [D, M_QK], bf16, kind="ExternalInput")
    wv_d = nc.dram_tensor("wv", [D, HL * A], bf16, kind="ExternalInput")
    bqk_d = nc.dram_tensor("bqk", [128, 4], f32, kind="ExternalInput")
    bv_d = nc.dram_tensor("bv", [1, HL * A], bf16, kind="ExternalInput")
    ones_d = nc.dram_tensor("ones", [1, 128], f32r, kind="ExternalInput")
    onesb_d = nc.dram_tensor("onesb", [1, 128], bf16, kind="ExternalInput")
    wout_d = nc.dram_tensor("wout", [128, 2, D], bf16, kind="ExternalInput")
    bout_d = nc.dram_tensor("bout", [1, D], f32, kind="ExternalInput")
    gamma_d = nc.dram_tensor("gamma", [1, D], f32, kind="ExternalInput")
    beta_d = nc.dram_tensor("beta", [1, D], f32, kind="ExternalInput")
    out_d = nc.dram_tensor("out", [QB, D], f32, kind="ExternalOutput")

    with tile.TileContext(nc) as tc:
        with (
            tc.tile_pool(name="big", bufs=1) as big,  # embT then maskT (64KB/p slot)
            tc.tile_pool(name="persist", bufs=1) as persist,
            tc.tile_pool(name="probs", bufs=2) as probsp,
            tc.tile_pool(name="work", bufs=2) as work,
            tc.tile_pool(name="psA", bufs=2, space="PSUM") as psA,  # 1-bank f32 mm
            tc.tile_pool(name="psS", bufs=2, space="PSUM") as psS,  # scores (2 banks)
            tc.tile_pool(name="psB", bufs=2, space="PSUM") as psB,  # pv xT (1 bank)
            tc.tile_pool(name="dram", bufs=1, space="DRAM") as dram,
        ):
            # ---------- embT first: it gates the QKV critical path ----------
            embT_sb = big.tile([128, 8, S], bf16, tag="bigslot")
            for kt in range(8):
                nc.sync.dma_start(out=embT_sb[:, kt, :], in_=embT_d[kt * 128 : (kt + 1) * 128, :])

            # ---------- weights / constants ----------
            wqk_sb = persist.tile([128, 8, M_QK], bf16)
            wv_sb = persist.tile([128, 8, HL * A], bf16)
            for kt in range(8):
                nc.sync.dma_start(out=wqk_sb[:, kt, :], in_=wqk_d[kt * 128 : (kt + 1) * 128, :])
                nc.sync.dma_start(out=wv_sb[:, kt, :], in_=wv_d[kt * 128 : (kt + 1) * 128, :])
            bqk_sb = persist.tile([128, 4], f32)
            nc.sync.dma_start(out=bqk_sb, in_=bqk_d[:, :])
            bv_sb = persist.tile([1, HL * A], bf16)
            nc.sync.dma_start(out=bv_sb, in_=bv_d[:, :])
            ones_b = persist.tile([1, 128], bf16)
            nc.sync.dma_start(out=ones_b, in_=onesb_d[:, :])
            ones64 = persist.tile([65, 128], f32r)
            nc.sync.dma_start(out=ones64[64:65, :], in_=ones_d[:, :])
            wout_sb = persist.tile([128, 2, D], bf16)
            nc.sync.dma_start(out=wout_sb, in_=wout_d[:, :, :])
            eps_sb = persist.tile([128, 1], f32)
            nc.vector.memset(eps_sb, EPS)
            boutbc = persist.tile([128, D], f32)
            gammabc = persist.tile([128, D], f32)
            betabc = persist.tile([128, D], f32)
            for t, dr in ((boutbc, bout_d), (gammabc, gamma_d), (betabc, beta_d)):
                src = dr[:, :]
                bc = bass.AP(tensor=src.tensor, offset=src.offset, ap=[[0, 128], src.ap[1]])
                nc.sync.dma_start(out=t[:], in_=bc)

            # ---------- QKV projection ----------
            # Q duplicated on both partition halves: q2[p, h, s], p<64 and
            # p>=64 both hold Q_h[p % 64, s].
            q2_sb = persist.tile([128, HL, S], bf16)
            # K block-diag: k2[0:64, h, kb, 0:64] = K_h[a, 128 kb + m],
            # k2[64:128, h, kb, 64:128] = K_h[a, 128 kb + 64 + m], zeros off-diag.
            k2_sb = persist.tile([128, HL, 16, 128], bf16)
            nc.vector.memset(k2_sb, 0.0)

            for mc in range(4):  # 0: Q h0-1, 1: Q h2-3, 2: K h0-1, 3: K h2-3
                for sc in range(4):
                    ps = psA.tile([128, 512], f32, tag="aux")
                    for kt in range(8):
                        nc.tensor.matmul(
                            ps[:],
                            wqk_sb[:, kt, mc * 128 : (mc + 1) * 128],
                            embT_sb[:, kt, sc * 512 : (sc + 1) * 512],
                            start=(kt == 0),
                            stop=(kt == 7),
                        )
                    if mc < 2:  # Q: natural-half eviction per head
                        he, ho = 2 * mc, 2 * mc + 1
                        nc.scalar.activation(
                            out=q2_sb[0:64, he, sc * 512 : (sc + 1) * 512],
                            in_=ps[0:64, :],
                            func=AF.Identity,
                            bias=bqk_sb[0:64, mc : mc + 1],
                            scale=1.0,
                        )
                        nc.scalar.activation(
                            out=q2_sb[64:128, ho, sc * 512 : (sc + 1) * 512],
                            in_=ps[64:128, :],
                            func=AF.Identity,
                            bias=bqk_sb[64:128, mc : mc + 1],
                            scale=1.0,
                        )
                    else:  # K: stage, then scatter into the diag blocks via DMA
                        he, ho = 2 * (mc - 2), 2 * (mc - 2) + 1
                        kstage = work.tile([128, 512], bf16, tag="kstage")
                        nc.scalar.activation(
                            out=kstage[:],
                            in_=ps[:],
                            func=AF.Identity,
                            bias=bqk_sb[:, mc : mc + 1],
                            scale=1.0,
                        )
                        ks = kstage.rearrange("p (k t m) -> p k t m", k=4, m=64)
                        kslc = slice(sc * 4, (sc + 1) * 4)
                        nc.sync.dma_start(
                            out=k2_sb[0:64, he, kslc, 0:64], in_=ks[0:64, :, 0, :]
                        )
                        nc.sync.dma_start(
                            out=k2_sb[64:128, he, kslc, 64:128], in_=ks[0:64, :, 1, :]
                        )
                        nc.sync.dma_start(
                            out=k2_sb[0:64, ho, kslc, 0:64], in_=ks[64:128, :, 0, :]
                        )
                        nc.sync.dma_start(
                            out=k2_sb[64:128, ho, kslc, 64:128], in_=ks[64:128, :, 1, :]
                        )
            # duplicate Q across partition halves (SBUF->SBUF DMA moves partitions)
            for h in range(HL):
                if h % 2 == 0:
                    nc.sync.dma_start(out=q2_sb[64:128, h, :], in_=q2_sb[0:64, h, :])
                else:
                    nc.sync.dma_start(out=q2_sb[0:64, h, :], in_=q2_sb[64:128, h, :])

            # V: [s, (h, a+1)] bf16, ones column LAST per head (sumexp row trick)
            v_sb = persist.tile([128, 16, HL, 1 + A], bf16)
            nc.vector.memset(v_sb, 1.0)
            for st in range(16):
                ps = psA.tile([128, HL * A], f32, tag="aux")
                for kt in range(8):
                    nc.tensor.matmul(
                        ps[:],
                        embT_sb[:, kt, st * 128 : (st + 1) * 128],
                        wv_sb[:, kt, :],
                        start=(kt == 0),
                        stop=False,
                    )
                nc.tensor.matmul(ps[:], ones_b[:, :], bv_sb[:, :], start=False, stop=True)
                nc.vector.tensor_copy(
                    out=v_sb[:, st, :, 0:A],
                    in_=ps.rearrange("p (h a) -> p h a", h=HL),
                )

            # xT: [(a), head pair, s] — heads stacked two per 128 partitions
            xT_sb = persist.tile([128, 2, S], bf16)
            rs_out = [
                dram.tile([128, D], bf16, name=f"rsout{q}", tag=f"rsout{q}")
                for q in range(4)
            ]

            # ---------- attention, software-pipelined two units deep ----------
            def pv_mms(pu, kb0, kb1):
                if pu["ps_x"] is None:
                    pu["ps_x"] = psB.tile([65, 512], f32, name="ps_x", tag="pvx")
                for kb in range(kb0, kb1):
                    nc.tensor.matmul(
                        pu["ps_x"][:],
                        v_sb[:, kb, pu["h"], :],
                        pu["probs"][:, kb, :],
                        start=(kb == 0),
                        stop=(kb == 15),
                    )

            def normalize_evict(pu):
                pq, ph, pps_x = pu["q"], pu["h"], pu["ps_x"]
                qo = pq * 512
                recip = work.tile([65, 512], f32r, tag="recip")
                with nc.allow_low_precision(reason="f32r is bitwise f32"):
                    nc.vector.reciprocal(recip[64:65, :], pps_x[64:65, :])
                ps_r = psA.tile([64, 512], f32, tag="aux")
                nc.tensor.matmul(
                    ps_r[:], ones64[64:65, 0:64], recip[64:65, :], start=True, stop=True
                )
                rb_sb = work.tile([64, 512], f32, tag="rbsb")
                nc.vector.tensor_copy(out=rb_sb[:], in_=ps_r[:])
                if ph % 2 == 0:
                    nc.vector.tensor_tensor(
                        xT_sb[0:64, ph // 2, qo : qo + 512],
                        pps_x[0:64, :],
                        rb_sb[:, :],
                        OP.mult,
                    )
                else:
                    xodd = work.tile([64, 512], bf16, tag="xodd")
                    nc.vector.tensor_tensor(
                        xodd[:], pps_x[0:64, :], rb_sb[:, :], OP.mult
                    )
                    nc.sync.dma_start(
                        out=xT_sb[64:128, ph // 2, qo : qo + 512], in_=xodd[:]
                    )

            def quarter_tail(q):
                qo = q * 512
                ar_in = dram.tile([QB, D], bf16, name=f"arin{q}", tag=f"arin{q}")
                for qc in range(4):
                    for dc in range(2):
                        ps_o = psA.tile([128, 512], f32, tag="aux")
                        for hp in range(2):
                            nc.tensor.matmul(
                                ps_o[:],
                                xT_sb[:, hp, qo + qc * 128 : qo + (qc + 1) * 128],
                                wout_sb[:, hp, dc * 512 : (dc + 1) * 512],
                                start=(hp == 0),
                                stop=(hp == 1),
                            )
                        oe = work.tile([128, 512], bf16, tag="oevict", bufs=3)
                        nc.any.tensor_copy(out=oe[:], in_=ps_o[:])
                        nc.sync.dma_start(
                            out=ar_in[qc * 128 : (qc + 1) * 128, dc * 512 : (dc + 1) * 512],
                            in_=oe[:],
                        )
                nc.gpsimd.collective_compute(
                    "ReduceScatter",
                    OP.add,
                    replica_groups=GROUPS,
                    ins=[ar_in[:, :].opt()],
                    outs=[rs_out[q][:, :].opt()],
                )
                # residual + LN on my 128 rows of this quarter
                rsl = slice(q * 128, (q + 1) * 128)
                rsb = work.tile([128, D], bf16, tag="rsb")
                nc.sync.dma_start(out=rsb[:], in_=rs_out[q][:, :])
                y = work.tile([128, D], f32, tag="y", bufs=1)
                er = work.tile([128, D], f32, tag="er", bufs=1)
                nc.sync.dma_start(out=er[:], in_=embres_d[rsl, :])
                nc.vector.tensor_tensor(y[:], er[:], rsb[:], OP.add)
                nc.vector.tensor_tensor(y[:], y[:], boutbc[:], OP.add)
                stats = work.tile([128, 2, nc.vector.BN_STATS_DIM], f32, tag="stats")
                for sg in range(2):
                    nc.vector.bn_stats(out=stats[:, sg, :], in_=y[:, sg * 512 : (sg + 1) * 512])
                mv = work.tile([128, nc.vector.BN_AGGR_DIM], f32, tag="mv")
                nc.vector.bn_aggr(out=mv[:], in_=stats[:])
                rstd = work.tile([128, 1], f32, tag="rstd")
                nc.scalar.activation(
                    out=rstd[:], in_=mv[:, 1:2], func=AF.Sqrt, bias=eps_sb[:], scale=1.0
                )
                nc.vector.reciprocal(rstd[:], rstd[:])
                nc.vector.tensor_scalar(
                    y[:], y[:], mv[:, 0:1], rstd[:], OP.subtract, OP.mult
                )
                o = work.tile([128, D], f32, tag="er", bufs=1)
                nc.vector.tensor_tensor(o[:], y[:], gammabc[:], OP.mult)
                nc.vector.tensor_tensor(o[:], o[:], betabc[:], OP.add)
                nc.sync.dma_start(out=out_d[rsl, :], in_=o[:])

            def finish(pu):
                normalize_evict(pu)
                if pu["h"] == 3:
                    quarter_tail(pu["q"])

            units = []
            mq = None
            for quarter in range(4):
                qoff = quarter * 512
                for h in range(4):
                    if h == 0:  # per-quarter mask slice, double-buffered
                        mq = work.tile([128, 16, 512], bf16, name="mq", tag="maskq")
                        for kb in range(16):
                            nc.sync.dma_start(
                                out=mq[:, kb, :],
                                in_=maskT_d[kb * 128 : (kb + 1) * 128, qoff : qoff + 512],
                            )
                    probs = probsp.tile([128, 16, 512], bf16, tag="probs")
                    unit = {"q": quarter, "h": h, "probs": probs, "ps_x": None, "mq": mq}
                    for j in range(8):  # kb pairs
                        ps_s = psS.tile([128, 2, 512], f32, tag="score")
                        for kk in range(2):
                            kb = 2 * j + kk
                            nc.tensor.matmul(
                                ps_s[:, kk, :],
                                k2_sb[:, h, kb, :],
                                q2_sb[:, h, qoff : qoff + 512],
                                start=True,
                                stop=True,
                            )
                        if units:
                            pv_mms(units[-1], 2 * j, 2 * j + 2)
                        nc.scalar.activation(
                            out=probs[:, 2 * j : 2 * j + 2, :],
                            in_=ps_s[:, :, :],
                            func=AF.Exp,
                            scale=0.125,
                        )
                        if j in (3, 7):  # mask applied in 8-kb batches
                            kb0 = 0 if j == 3 else 8
                            nc.vector.tensor_tensor(
                                probs[:, kb0 : kb0 + 8, :],
                                probs[:, kb0 : kb0 + 8, :],
                                mq[:, kb0 : kb0 + 8, :],
                                OP.mult,
                            )
                        if j == 1 and len(units) >= 2:
                            finish(units[-2])
                    units.append(unit)
            finish(units[-2])
            pv_mms(units[-1], 0, 16)
            finish(units[-1])

    nc.compile()
    return nc


def _prep_inputs(embeddings, attention_mask, W_qkv, b_qkv, W_out, b_out, ln_gamma, ln_beta):
    emb = np.asarray(embeddings, dtype=np.float32)
    mask = np.asarray(attention_mask)
    W_qkv = np.asarray(W_qkv, dtype=np.float32)
    b_qkv = np.asarray(b_qkv, dtype=np.float32)
    W_out = np.asarray(W_out, dtype=np.float32)
    b_out = np.asarray(b_out, dtype=np.float32)
    gamma = np.asarray(ln_gamma, dtype=np.float32).reshape(1, D)
    beta = np.asarray(ln_beta, dtype=np.float32).reshape(1, D)

    in_maps = []
    for c in range(NCORES):
        b = c // G
        g = c % G
        hs = g * HL * A  # 256g
        embT = np.ascontiguousarray(emb[b].T).astype(ml_dtypes.bfloat16)
        maskT = np.ascontiguousarray(mask[b].T).astype(ml_dtypes.bfloat16)
        wqk = np.ascontiguousarray(
            np.concatenate([W_qkv[:, hs : hs + 256], W_qkv[:, D + hs : D + hs + 256]], axis=1)
        ).astype(ml_dtypes.bfloat16)
        wv = np.ascontiguousarray(W_qkv[:, 2 * D + hs : 2 * D + hs + 256]).astype(
            ml_dtypes.bfloat16
        )
        bqk = np.concatenate([b_qkv[hs : hs + 256], b_qkv[D + hs : D + hs + 256]])
        bqk = np.ascontiguousarray(bqk.reshape(4, 128).T)
        bv = np.ascontiguousarray(
            b_qkv[2 * D + hs : 2 * D + hs + 256].reshape(1, 256)
        ).astype(ml_dtypes.bfloat16)
        wout = np.ascontiguousarray(
            W_out[hs : hs + 256, :].reshape(2, 128, D).transpose(1, 0, 2)
        ).astype(ml_dtypes.bfloat16)
        embres = np.concatenate(
            [emb[b, 512 * q + 128 * g : 512 * q + 128 * g + 128, :] for q in range(4)],
            axis=0,
        )
        in_maps.append(
            {
                "embT": embT,
                "embres": np.ascontiguousarray(embres),
                "maskT": maskT,
                "wqk": wqk,
                "wv": wv,
                "bqk": bqk,
                "bv": bv,
                "ones": np.ones((1, 128), dtype=np.float32),
                "onesb": np.ones((1, 128), dtype=ml_dtypes.bfloat16),
                "wout": wout,
                "bout": b_out.reshape(1, D),
                "gamma": gamma,
                "beta": beta,
            }
        )
    return in_maps


def _run(inputs, trace=False, **kw):
    if "nc" not in _CACHE:
        _CACHE["nc"] = _build()
    nc = _CACHE["nc"]
    in_maps = _prep_inputs(**inputs)
    res = run_bass_kernel_spmd(nc, in_maps, list(range(NCORES)), trace=trace, **kw)
    out = np.empty((B, S, D), dtype=np.float32)
    for c in range(NCORES):
        b, g = c // G, c % G
        for q in range(4):
            out[b, 512 * q + 128 * g : 512 * q + 128 * g + 128, :] = res.results[c][
                "out"
            ][128 * q : 128 * (q + 1), :]
    return out, res


def kernel(**inputs):
    out, _ = _run(inputs, trace=False)
    return out
